# revision 41
# baseline (speedup 1.0000x reference)
"""IntraSentenceGNN (3-node GATv2 x2) Trainium2 kernel.

Full inputs in, full outputs out; batch (32768) is sharded over 8 NeuronCores
(pure data parallel, weights replicated). Math:
    t = text @ Wt + bt ; a = audio @ Wa + ba ; v = video @ Wv + bv
    x = stack([t,a,v])                       # [B, 3, H]
    x = elu(gatv2(x, Wl1, Wr1, att1, b1, heads=8, concat))
    x = gatv2(x, Wl2, Wr2, att2, b2, heads=1, mean)
    out = x.mean(nodes)                      # [B, H]

Key restructurings (host-side, exact):
  - Projection fused into layer-1 weights: gl1_t = text @ (Wt @ Wl1) + (bt@Wl1)
    (and 5 more), removing the 3 projection matmuls entirely.
  - Final node-mean folded into softmax coefs: out = sum_j (mean_i alpha2_ij) gl2_j.
  - Features pre-transposed on host so X^T chunks act as matmul stationaries.
"""
import os, sys

os.environ.setdefault("JAX_PLATFORMS", "cpu")
for _p in ("/opt/trn_rl_repo", "/root/.axon_site/_ro/trn_rl_repo"):
    if _p not in sys.path and os.path.isdir(_p):
        sys.path.append(_p)

import numpy as np
import ml_dtypes

import concourse.bass as bass
import concourse.bacc as bacc
import concourse.mybir as mybir
from concourse.tile import TileContext
from concourse.bass_utils import run_bass_kernel_spmd
from concourse.masks import make_identity
from concourse.dve_ops import TENSOR_TENSOR_REDUCE as CTTR
import concourse.dve_ops as _dve_ops
from concourse.dve_spec import Spec as _Spec, Src0 as _S0, Src1 as _S1, \
    C0 as _C0, C1 as _C1, C2 as _C2, maxx as _maxx
from operator import add as _addop


def _register_lrelu_ttr():
    """Custom DVE op: out = max(z, z*c1)*att*c2 ; accum = c0 + sum(out).
    Fuses leaky-relu + attention weighting + reduce into one DVE pass."""
    name = "LRELU_TT_REDUCE"
    for op in _dve_ops.OPS:
        if op.name == name:
            return op
    import numpy as _np
    spec = _Spec(
        body=_maxx(_S0, _S0 * _C1) * _S1 * _C2,
        accum=_addop,
        accum_init=_C0,
        reference=_dve_ops._ref_body_sum(
            lambda in0, in1, c0, c1, c2: _np.maximum(
                in0.astype(_np.float32), in0.astype(_np.float32) * c1)
            * in1 * c2),
    )
    op = _dve_ops.DveOp(name, spec, subdim=False,
                        uops_sha={"v3": "42c60a78ac67794a",
                                  "v4": "a12d78378010f3eb"})
    _dve_ops.OPS.append(op)
    _dve_ops._SUB_OPCODE_FOR_NAME[name] = (
        _dve_ops._CUSTOM_DVE_ROW_BASE + len(_dve_ops.OPS) - 1)
    _dve_ops.CUSTOM_DVE_SPECS[name] = spec
    return op


LRELU_TTR = _register_lrelu_ttr()

F32 = mybir.dt.float32
BF16 = mybir.dt.bfloat16
AX = mybir.AxisListType
OP = mybir.AluOpType
AF = mybir.ActivationFunctionType

B = 32768
D = 1024
H = 1024
HEADS = 8
DH = 128
N = 3
NCORES = 8

# --- engine-assignment flags (perf tuning) ---
USE_ACT_LRELU = True     # leaky-relu on ScalarE (hardware Lrelu LUT)
Z_ON_GPSIMD = False       # z = gl_j + gr_i adds on GpSimd (walrus rejects TT on Pool)
COMBINE1_GPSIMD = False   # combine1 alpha-scaling on GpSimd tensor_scalar
L1_TTR_NARROW = True     # L1 per-head fused mult+reduce (8 narrow TTRs)

_CACHED = {}


def _build_nc(R):
    NT = R // 128
    nc = bacc.Bacc(None)

    xt = [nc.declare_dram_parameter(f"xt{j}", [NT, 128, 8, 128], BF16,
                                    isOutput=False)
          for j in range(N)]
    wgl = [nc.declare_dram_parameter(f"wgl{j}", [128, 8, H], BF16,
                                     isOutput=False)
           for j in range(N)]
    wgr = [nc.declare_dram_parameter(f"wgr{j}", [128, 8, H], BF16,
                                     isOutput=False)
           for j in range(N)]
    wl2 = nc.declare_dram_parameter("wl2", [128, 8, H], BF16, isOutput=False)
    wr2 = nc.declare_dram_parameter("wr2", [128, 8, H], BF16, isOutput=False)
    att1_in = nc.declare_dram_parameter("att1", [1, H], BF16, isOutput=False)
    att2_in = nc.declare_dram_parameter("att2", [1, H], BF16, isOutput=False)
    out_d = nc.declare_dram_parameter("out", [R, H], F32, isOutput=True)

    from contextlib import ExitStack
    with TileContext(nc) as tc, ExitStack() as es:
        pool = lambda **kw: es.enter_context(tc.tile_pool(**kw))
        consts = pool(name="consts", bufs=1)
        wpool = pool(name="wpool", bufs=1)
        xpool = pool(name="xpool", bufs=2)
        glgr = pool(name="glgr", bufs=12)
        x2pool = pool(name="x2pool", bufs=3)
        x2tpool = pool(name="x2tpool", bufs=3)
        scratch = pool(name="scratch", bufs=2)
        smalls = pool(name="smalls", bufs=6)
        outpool = pool(name="outpool", bufs=2)
        psmm = pool(name="psmm", bufs=3, space="PSUM")
        pstp = pool(name="pstp", bufs=2, space="PSUM")
        if True:
            # constants
            attB1 = consts.tile([128, H], BF16)
            nc.sync.dma_start(out=attB1, in_=att1_in[:].broadcast_to([128, H]))
            attB2 = consts.tile([128, H], BF16)
            nc.sync.dma_start(out=attB2, in_=att2_in[:].broadcast_to([128, H]))
            ident = consts.tile([128, 128], BF16)
            make_identity(nc, ident)

            # resident weights: [128, 8, 1024] (partition = d within chunk)
            def load_w(dram):
                t = wpool.tile([128, 8, H], BF16, tag=f"w_{dram.name}",
                               name=f"w_{dram.name}")
                nc.sync.dma_start(out=t, in_=dram[:])
                return t
            WGl = [load_w(w) for w in wgl]
            WGr = [load_w(w) for w in wgr]
            WL2 = load_w(wl2)
            WR2 = load_w(wr2)

            def mm(ps, xT, w):
                # ps[128b, 1024] += xT[:, k, :].T @ w[:, k, :] over k
                for k in range(8):
                    for nh in range(2):
                        nc.tensor.matmul(
                            ps[:, nh * 512:(nh + 1) * 512],
                            xT[:, k, :], w[:, k, nh * 512:(nh + 1) * 512],
                            start=(k == 0), stop=(k == 7))

            def lrelu(dst, src):
                if USE_ACT_LRELU:
                    nc.scalar.activation(dst, src, AF.Prelu, alpha=0.2)
                else:
                    t = scratch.tile([128, H], BF16, tag="lrelu_t")
                    nc.vector.tensor_scalar_mul(t, src, 0.2)
                    nc.vector.tensor_tensor(out=dst, in0=src, in1=t, op=OP.max)

            eng_z = nc.gpsimd if Z_ON_GPSIMD else nc.vector

            for it in range(NT):
                bs = bass.ts(it, 128)
                # --- load transposed feature tiles ---
                xts = []
                for j in range(N):
                    t = xpool.tile([128, 8, 128], BF16, tag=f"xt{j}")
                    nc.sync.dma_start(out=t, in_=xt[j][it])
                    xts.append(t)

                # --- layer 1 matmuls ---
                gl1, gr1 = [], []
                for j in range(N):
                    for lst, w in ((gl1, WGl[j]), (gr1, WGr[j])):
                        ps = psmm.tile([128, H], F32, tag="mm")
                        mm(ps, xts[j], w)
                        sb = glgr.tile([128, H], BF16, tag="glgr")
                        nc.scalar.copy(sb, ps)
                        lst.append(sb)

                # --- layer 1 attention scores ---
                # E[i]: [128, 3, 8] fp32 ; e_ij_h = sum_d att1*lrelu(gl_j+gr_i)
                E = [smalls.tile([128, N, HEADS], F32, tag=f"E{i}",
                                 name=f"E{i}_{it}")
                     for i in range(N)]
                for i in range(N):
                    for j in range(N):
                        z = scratch.tile([128, H], BF16, tag="z")
                        eng_z.tensor_tensor(out=z, in0=gl1[j], in1=gr1[i],
                                            op=OP.add)
                        mw = scratch.tile([128, H], BF16, tag="mw")
                        for h in range(HEADS):
                            hs = bass.ts(h, DH)
                            nc.vector._custom_dve(
                                LRELU_TTR, out=mw[:, hs], in0=z[:, hs],
                                in1=attB1[:, hs], s0=0.0, s1=0.2, imm2=1.0,
                                accum_out=E[i][:, j, h:h + 1])

                # --- layer 1 softmax over j + combine + elu -> x2 ---
                x2 = []
                for i in range(N):
                    w = smalls.tile([128, N, HEADS], F32, tag="w1")
                    nc.scalar.activation(w, E[i], AF.Exp)
                    s = smalls.tile([128, HEADS], F32, tag="s1")
                    # sum over j (stride 8 within [3,8] layout)
                    nc.vector.tensor_reduce(
                        out=s, in_=w.rearrange("p j h -> p h j"),
                        axis=AX.X, op=OP.add)
                    r = smalls.tile([128, HEADS], F32, tag="r1")
                    nc.vector.reciprocal(r, s)
                    alp = smalls.tile([128, N, HEADS], F32, tag="alp1")
                    nc.vector.tensor_tensor(
                        out=alp, in0=w,
                        in1=r.rearrange("p (o h) -> p o h", o=1).broadcast_to(
                            [128, N, HEADS]),
                        op=OP.mult)


                    acc = x2pool.tile([128, H], BF16, tag="x2")
                    for h in range(HEADS):
                        hs = bass.ts(h, DH)
                        nc.vector.tensor_scalar_mul(
                            acc[:, hs], gl1[0][:, hs], alp[:, 0, h:h + 1])
                        for j in (1, 2):
                            nc.vector.scalar_tensor_tensor(
                                out=acc[:, hs], in0=gl1[j][:, hs],
                                scalar=alp[:, j, h:h + 1], in1=acc[:, hs],
                                op0=OP.mult, op1=OP.add)
                    # elu(acc) = max(acc, exp(min(acc,0)) - 1)
                    mn = scratch.tile([128, H], BF16, tag="z")
                    nc.vector.tensor_scalar_min(mn, acc, 0.0)
                    ex = scratch.tile([128, H], BF16, tag="mw")
                    nc.scalar.activation(ex, mn, AF.Exp)
                    d = scratch.tile([128, H], BF16, tag="z")
                    nc.vector.tensor_scalar_add(d, ex, -1.0)
                    xi = x2pool.tile([128, H], BF16, tag="x2e")
                    nc.vector.tensor_tensor(out=xi, in0=acc, in1=d, op=OP.max)
                    x2.append(xi)

                # --- transpose x2 (PE) -> x2T [128, 8, 128] ---
                x2T = []
                for j in range(N):
                    pst = pstp.tile([128, 8, 128], BF16, tag="tp")
                    for c in range(8):
                        nc.tensor.transpose(
                            pst[:, c, :], x2[j][:, bass.ts(c, 128)], ident)
                    sb = x2tpool.tile([128, 8, 128], BF16, tag="x2t",
                                      name=f"x2t{j}_{it}")
                    nc.scalar.copy(sb, pst)
                    x2T.append(sb)

                # --- layer 2 matmuls ---
                gl2, gr2 = [], []
                for j in range(N):
                    for lst, w in ((gl2, WL2), (gr2, WR2)):
                        ps = psmm.tile([128, H], F32, tag="mm")
                        mm(ps, x2T[j], w)
                        sb = glgr.tile([128, H], BF16, tag="glgr")
                        nc.scalar.copy(sb, ps)
                        lst.append(sb)

                # --- layer 2 scores: E2 [128, 3, 3] (i, j) ---
                E2 = smalls.tile([128, N, N], F32, tag="E2")
                for i in range(N):
                    for j in range(N):
                        z = scratch.tile([128, H], BF16, tag="z")
                        eng_z.tensor_tensor(out=z, in0=gl2[j], in1=gr2[i],
                                            op=OP.add)
                        mw = scratch.tile([128, H], BF16, tag="mw")
                        nc.vector._custom_dve(
                            LRELU_TTR, out=mw, in0=z, in1=attB2,
                            s0=0.0, s1=0.2, imm2=1.0,
                            accum_out=E2[:, i, j:j + 1])

                # softmax over j, then c_j = mean_i alpha2_ij / 3
                w2 = smalls.tile([128, N, N], F32, tag="w2")
                nc.scalar.activation(w2, E2, AF.Exp)
                s2 = smalls.tile([128, N], F32, tag="s2")
                nc.vector.tensor_reduce(out=s2, in_=w2, axis=AX.X, op=OP.add)
                r2 = smalls.tile([128, N], F32, tag="r2")
                nc.vector.reciprocal(r2, s2)
                al2 = smalls.tile([128, N, N], F32, tag="al2")
                nc.vector.tensor_tensor(
                    out=al2, in0=w2,
                    in1=r2.rearrange("p (i o) -> p i o", o=1).broadcast_to([128, N, N]),
                    op=OP.mult)
                c0 = smalls.tile([128, N], F32, tag="c0")
                nc.vector.tensor_tensor(out=c0, in0=al2[:, 0, :],
                                        in1=al2[:, 1, :], op=OP.add)
                c = smalls.tile([128, N], F32, tag="c")
                nc.vector.scalar_tensor_tensor(
                    out=c, in0=c0, scalar=1.0, in1=al2[:, 2, :],
                    op0=OP.mult, op1=OP.add)
                nc.vector.tensor_scalar_mul(c, c, 1.0 / 3.0)

                # --- combine2 -> out tile fp32 ---
                ot = outpool.tile([128, H], F32, tag="out")
                nc.vector.tensor_scalar_mul(ot, gl2[0], c[:, 0:1])
                for j in (1, 2):
                    nc.vector.scalar_tensor_tensor(
                        out=ot, in0=gl2[j], scalar=c[:, j:j + 1], in1=ot,
                        op0=OP.mult, op1=OP.add)
                nc.sync.dma_start(out=out_d[bs, :], in_=ot)

    nc.finalize()
    return nc


def _prep(inputs, R):
    f32 = np.float32
    bf = ml_dtypes.bfloat16
    g = lambda k: np.asarray(inputs[k], f32)
    Wl1, Wr1 = g("Wl1"), g("Wr1")
    feats = [g("text_features"), g("audio_features"), g("video_features")]
    Ws = [g("Wt"), g("Wa"), g("Wv")]
    biases = [g("bt"), g("ba"), g("bv"), g("b1"), g("b2")]
    if any(np.any(b) for b in biases):
        raise NotImplementedError("nonzero biases not supported by this kernel")

    def wlay(w):
        # [1024, H] -> [128(p), 8(c), H] with row (c*128+p) at [p, c]
        return np.ascontiguousarray(
            w.reshape(8, 128, H).transpose(1, 0, 2).astype(bf))

    shared = {}
    for j in range(N):
        shared[f"wgl{j}"] = wlay(Ws[j] @ Wl1)
        shared[f"wgr{j}"] = wlay(Ws[j] @ Wr1)
    shared["wl2"] = wlay(g("Wl2"))
    shared["wr2"] = wlay(g("Wr2"))
    shared["att1"] = g("att1").reshape(1, H).astype(bf)
    shared["att2"] = g("att2").reshape(1, H).astype(bf)

    in_maps = []
    for core in range(NCORES):
        sl = slice(core * R, (core + 1) * R)
        m = dict(shared)
        NT = R // 128
        for j in range(N):
            # [R, D] -> [NT, 128(p=d%128), 8(c=d//128), 128(b)]
            a = feats[j][sl].reshape(NT, 128, 8, 128)
            m[f"xt{j}"] = np.ascontiguousarray(
                a.transpose(0, 3, 2, 1).astype(bf))
        in_maps.append(m)
    return in_maps


def kernel(**inputs):
    Btot = int(np.asarray(inputs["text_features"]).shape[0])
    assert Btot % (NCORES * 128) == 0, f"batch {Btot} not divisible by 1024"
    R = Btot // NCORES
    if R not in _CACHED:
        _CACHED[R] = _build_nc(R)
    nc = _CACHED[R]
    in_maps = _prep(inputs, R)
    res = run_bass_kernel_spmd(nc, in_maps, core_ids=list(range(NCORES)))
    _CACHED["last_result"] = res
    out = np.concatenate([r["out"] for r in res.results], axis=0)
    return np.ascontiguousarray(out.astype(np.float32))


# revision 42
# speedup vs baseline: 1.0024x; 1.0024x over previous
"""IntraSentenceGNN (3-node GATv2 x2) Trainium2 kernel.

Full inputs in, full outputs out; batch (32768) is sharded over 8 NeuronCores
(pure data parallel, weights replicated). Math:
    t = text @ Wt + bt ; a = audio @ Wa + ba ; v = video @ Wv + bv
    x = stack([t,a,v])                       # [B, 3, H]
    x = elu(gatv2(x, Wl1, Wr1, att1, b1, heads=8, concat))
    x = gatv2(x, Wl2, Wr2, att2, b2, heads=1, mean)
    out = x.mean(nodes)                      # [B, H]

Key restructurings (host-side, exact):
  - Projection fused into layer-1 weights: gl1_t = text @ (Wt @ Wl1) + (bt@Wl1)
    (and 5 more), removing the 3 projection matmuls entirely.
  - Final node-mean folded into softmax coefs: out = sum_j (mean_i alpha2_ij) gl2_j.
  - Features pre-transposed on host so X^T chunks act as matmul stationaries.
"""
import os, sys

os.environ.setdefault("JAX_PLATFORMS", "cpu")
for _p in ("/opt/trn_rl_repo", "/root/.axon_site/_ro/trn_rl_repo"):
    if _p not in sys.path and os.path.isdir(_p):
        sys.path.append(_p)

import numpy as np
import ml_dtypes

import concourse.bass as bass
import concourse.bacc as bacc
import concourse.mybir as mybir
from concourse.tile import TileContext
from concourse.bass_utils import run_bass_kernel_spmd
from concourse.masks import make_identity
from concourse.dve_ops import TENSOR_TENSOR_REDUCE as CTTR
import concourse.dve_ops as _dve_ops
from concourse.dve_spec import Spec as _Spec, Src0 as _S0, Src1 as _S1, \
    C0 as _C0, C1 as _C1, C2 as _C2, maxx as _maxx
from operator import add as _addop


def _register_lrelu_ttr():
    """Custom DVE op: out = max(z, z*c1)*att*c2 ; accum = c0 + sum(out).
    Fuses leaky-relu + attention weighting + reduce into one DVE pass."""
    name = "LRELU_TT_REDUCE"
    for op in _dve_ops.OPS:
        if op.name == name:
            return op
    import numpy as _np
    spec = _Spec(
        body=_maxx(_S0, _S0 * _C1) * _S1 * _C2,
        accum=_addop,
        accum_init=_C0,
        reference=_dve_ops._ref_body_sum(
            lambda in0, in1, c0, c1, c2: _np.maximum(
                in0.astype(_np.float32), in0.astype(_np.float32) * c1)
            * in1 * c2),
    )
    op = _dve_ops.DveOp(name, spec, subdim=False,
                        uops_sha={"v3": "42c60a78ac67794a",
                                  "v4": "a12d78378010f3eb"})
    _dve_ops.OPS.append(op)
    _dve_ops._SUB_OPCODE_FOR_NAME[name] = (
        _dve_ops._CUSTOM_DVE_ROW_BASE + len(_dve_ops.OPS) - 1)
    _dve_ops.CUSTOM_DVE_SPECS[name] = spec
    return op


LRELU_TTR = _register_lrelu_ttr()

F32 = mybir.dt.float32
BF16 = mybir.dt.bfloat16
AX = mybir.AxisListType
OP = mybir.AluOpType
AF = mybir.ActivationFunctionType

B = 32768
D = 1024
H = 1024
HEADS = 8
DH = 128
N = 3
NCORES = 8

# --- engine-assignment flags (perf tuning) ---
USE_ACT_LRELU = True     # leaky-relu on ScalarE (hardware Lrelu LUT)
Z_ON_GPSIMD = False       # z = gl_j + gr_i adds on GpSimd (walrus rejects TT on Pool)
COMBINE1_GPSIMD = False   # combine1 alpha-scaling on GpSimd tensor_scalar
L1_TTR_NARROW = True     # L1 per-head fused mult+reduce (8 narrow TTRs)

_CACHED = {}


def _build_nc(R):
    NT = R // 128
    nc = bacc.Bacc(None)

    xt = [nc.declare_dram_parameter(f"xt{j}", [NT, 128, 8, 128], BF16,
                                    isOutput=False)
          for j in range(N)]
    wgl = [nc.declare_dram_parameter(f"wgl{j}", [128, 8, H], BF16,
                                     isOutput=False)
           for j in range(N)]
    wgr = [nc.declare_dram_parameter(f"wgr{j}", [128, 8, H], BF16,
                                     isOutput=False)
           for j in range(N)]
    wl2 = nc.declare_dram_parameter("wl2", [128, 8, H], BF16, isOutput=False)
    wr2 = nc.declare_dram_parameter("wr2", [128, 8, H], BF16, isOutput=False)
    att1_in = nc.declare_dram_parameter("att1", [1, H], BF16, isOutput=False)
    att2_in = nc.declare_dram_parameter("att2", [1, H], BF16, isOutput=False)
    out_d = nc.declare_dram_parameter("out", [R, H], F32, isOutput=True)

    from contextlib import ExitStack
    with TileContext(nc) as tc, ExitStack() as es:
        pool = lambda **kw: es.enter_context(tc.tile_pool(**kw))
        consts = pool(name="consts", bufs=1)
        wpool = pool(name="wpool", bufs=1)
        xpool = pool(name="xpool", bufs=2)
        glgr = pool(name="glgr", bufs=12)
        x2pool = pool(name="x2pool", bufs=3)
        x2tpool = pool(name="x2tpool", bufs=4)
        scratch = pool(name="scratch", bufs=2)
        smalls = pool(name="smalls", bufs=4)
        outpool = pool(name="outpool", bufs=2)
        psmm = pool(name="psmm", bufs=3, space="PSUM")
        pstp = pool(name="pstp", bufs=2, space="PSUM")
        if True:
            # constants
            attB1 = consts.tile([128, H], BF16)
            nc.sync.dma_start(out=attB1, in_=att1_in[:].broadcast_to([128, H]))
            attB2 = consts.tile([128, H], BF16)
            nc.sync.dma_start(out=attB2, in_=att2_in[:].broadcast_to([128, H]))
            ident = consts.tile([128, 128], BF16)
            make_identity(nc, ident)

            # resident weights: [128, 8, 1024] (partition = d within chunk)
            def load_w(dram):
                t = wpool.tile([128, 8, H], BF16, tag=f"w_{dram.name}",
                               name=f"w_{dram.name}")
                nc.sync.dma_start(out=t, in_=dram[:])
                return t
            WGl = [load_w(w) for w in wgl]
            WGr = [load_w(w) for w in wgr]
            WL2 = load_w(wl2)
            WR2 = load_w(wr2)

            def mm(ps, xT, w):
                # ps[128b, 1024] += xT[:, k, :].T @ w[:, k, :] over k
                for k in range(8):
                    for nh in range(2):
                        nc.tensor.matmul(
                            ps[:, nh * 512:(nh + 1) * 512],
                            xT[:, k, :], w[:, k, nh * 512:(nh + 1) * 512],
                            start=(k == 0), stop=(k == 7))

            def lrelu(dst, src):
                if USE_ACT_LRELU:
                    nc.scalar.activation(dst, src, AF.Prelu, alpha=0.2)
                else:
                    t = scratch.tile([128, H], BF16, tag="lrelu_t")
                    nc.vector.tensor_scalar_mul(t, src, 0.2)
                    nc.vector.tensor_tensor(out=dst, in0=src, in1=t, op=OP.max)

            eng_z = nc.gpsimd if Z_ON_GPSIMD else nc.vector

            for it in range(NT):
                bs = bass.ts(it, 128)
                # --- load transposed feature tiles ---
                xts = []
                for j in range(N):
                    t = xpool.tile([128, 8, 128], BF16, tag=f"xt{j}")
                    nc.sync.dma_start(out=t, in_=xt[j][it])
                    xts.append(t)

                # --- layer 1 matmuls ---
                gl1, gr1 = [], []
                for j in range(N):
                    for lst, w in ((gl1, WGl[j]), (gr1, WGr[j])):
                        ps = psmm.tile([128, H], F32, tag="mm")
                        mm(ps, xts[j], w)
                        sb = glgr.tile([128, H], BF16, tag="glgr")
                        nc.scalar.copy(sb, ps)
                        lst.append(sb)

                # --- layer 1 attention scores ---
                # E[i]: [128, 3, 8] fp32 ; e_ij_h = sum_d att1*lrelu(gl_j+gr_i)
                E = [smalls.tile([128, N, HEADS], F32, tag=f"E{i}",
                                 name=f"E{i}_{it}")
                     for i in range(N)]
                for i in range(N):
                    for j in range(N):
                        z = scratch.tile([128, H], BF16, tag="z")
                        eng_z.tensor_tensor(out=z, in0=gl1[j], in1=gr1[i],
                                            op=OP.add)
                        mw = scratch.tile([128, H], BF16, tag="mw")
                        for h in range(HEADS):
                            hs = bass.ts(h, DH)
                            nc.vector._custom_dve(
                                LRELU_TTR, out=mw[:, hs], in0=z[:, hs],
                                in1=attB1[:, hs], s0=0.0, s1=0.2, imm2=1.0,
                                accum_out=E[i][:, j, h:h + 1])

                # --- layer 1 softmax over j + combine + elu -> x2 ---
                x2 = []
                for i in range(N):
                    w = smalls.tile([128, N, HEADS], F32, tag="w1")
                    nc.scalar.activation(w, E[i], AF.Exp)
                    s = smalls.tile([128, HEADS], F32, tag="s1")
                    # sum over j (stride 8 within [3,8] layout)
                    nc.vector.tensor_reduce(
                        out=s, in_=w.rearrange("p j h -> p h j"),
                        axis=AX.X, op=OP.add)
                    r = smalls.tile([128, HEADS], F32, tag="r1")
                    nc.vector.reciprocal(r, s)
                    alp = smalls.tile([128, N, HEADS], F32, tag="alp1")
                    nc.vector.tensor_tensor(
                        out=alp, in0=w,
                        in1=r.rearrange("p (o h) -> p o h", o=1).broadcast_to(
                            [128, N, HEADS]),
                        op=OP.mult)


                    acc = x2pool.tile([128, H], BF16, tag="x2")
                    for h in range(HEADS):
                        hs = bass.ts(h, DH)
                        nc.vector.tensor_scalar_mul(
                            acc[:, hs], gl1[0][:, hs], alp[:, 0, h:h + 1])
                        for j in (1, 2):
                            nc.vector.scalar_tensor_tensor(
                                out=acc[:, hs], in0=gl1[j][:, hs],
                                scalar=alp[:, j, h:h + 1], in1=acc[:, hs],
                                op0=OP.mult, op1=OP.add)
                    # elu(acc) = max(acc, exp(min(acc,0)) - 1)
                    mn = scratch.tile([128, H], BF16, tag="z")
                    nc.vector.tensor_scalar_min(mn, acc, 0.0)
                    ex = scratch.tile([128, H], BF16, tag="mw")
                    nc.scalar.activation(ex, mn, AF.Exp)
                    d = scratch.tile([128, H], BF16, tag="z")
                    nc.vector.tensor_scalar_add(d, ex, -1.0)
                    xi = x2pool.tile([128, H], BF16, tag="x2e")
                    nc.vector.tensor_tensor(out=xi, in0=acc, in1=d, op=OP.max)
                    x2.append(xi)

                # --- transpose x2 (PE) -> x2T [128, 8, 128] ---
                x2T = []
                for j in range(N):
                    pst = pstp.tile([128, 8, 128], BF16, tag="tp")
                    for c in range(8):
                        nc.tensor.transpose(
                            pst[:, c, :], x2[j][:, bass.ts(c, 128)], ident)
                    sb = x2tpool.tile([128, 8, 128], BF16, tag="x2t",
                                      name=f"x2t{j}_{it}")
                    nc.scalar.copy(sb, pst)
                    x2T.append(sb)

                # --- layer 2 matmuls ---
                gl2, gr2 = [], []
                for j in range(N):
                    for lst, w in ((gl2, WL2), (gr2, WR2)):
                        ps = psmm.tile([128, H], F32, tag="mm")
                        mm(ps, x2T[j], w)
                        sb = glgr.tile([128, H], BF16, tag="glgr")
                        nc.scalar.copy(sb, ps)
                        lst.append(sb)

                # --- layer 2 scores: E2 [128, 3, 3] (i, j) ---
                E2 = smalls.tile([128, N, N], F32, tag="E2")
                for i in range(N):
                    for j in range(N):
                        z = scratch.tile([128, H], BF16, tag="z")
                        eng_z.tensor_tensor(out=z, in0=gl2[j], in1=gr2[i],
                                            op=OP.add)
                        mw = scratch.tile([128, H], BF16, tag="mw")
                        nc.vector._custom_dve(
                            LRELU_TTR, out=mw, in0=z, in1=attB2,
                            s0=0.0, s1=0.2, imm2=1.0,
                            accum_out=E2[:, i, j:j + 1])

                # softmax over j, then c_j = mean_i alpha2_ij / 3
                w2 = smalls.tile([128, N, N], F32, tag="w2")
                nc.scalar.activation(w2, E2, AF.Exp)
                s2 = smalls.tile([128, N], F32, tag="s2")
                nc.vector.tensor_reduce(out=s2, in_=w2, axis=AX.X, op=OP.add)
                r2 = smalls.tile([128, N], F32, tag="r2")
                nc.vector.reciprocal(r2, s2)
                al2 = smalls.tile([128, N, N], F32, tag="al2")
                nc.vector.tensor_tensor(
                    out=al2, in0=w2,
                    in1=r2.rearrange("p (i o) -> p i o", o=1).broadcast_to([128, N, N]),
                    op=OP.mult)
                c0 = smalls.tile([128, N], F32, tag="c0")
                nc.vector.tensor_tensor(out=c0, in0=al2[:, 0, :],
                                        in1=al2[:, 1, :], op=OP.add)
                c = smalls.tile([128, N], F32, tag="c")
                nc.vector.scalar_tensor_tensor(
                    out=c, in0=c0, scalar=1.0, in1=al2[:, 2, :],
                    op0=OP.mult, op1=OP.add)
                nc.vector.tensor_scalar_mul(c, c, 1.0 / 3.0)

                # --- combine2 -> out tile fp32 ---
                ot = outpool.tile([128, H], F32, tag="out")
                nc.vector.tensor_scalar_mul(ot, gl2[0], c[:, 0:1])
                for j in (1, 2):
                    nc.vector.scalar_tensor_tensor(
                        out=ot, in0=gl2[j], scalar=c[:, j:j + 1], in1=ot,
                        op0=OP.mult, op1=OP.add)
                nc.sync.dma_start(out=out_d[bs, :], in_=ot)

    nc.finalize()
    return nc


def _prep(inputs, R):
    f32 = np.float32
    bf = ml_dtypes.bfloat16
    g = lambda k: np.asarray(inputs[k], f32)
    Wl1, Wr1 = g("Wl1"), g("Wr1")
    feats = [g("text_features"), g("audio_features"), g("video_features")]
    Ws = [g("Wt"), g("Wa"), g("Wv")]
    biases = [g("bt"), g("ba"), g("bv"), g("b1"), g("b2")]
    if any(np.any(b) for b in biases):
        raise NotImplementedError("nonzero biases not supported by this kernel")

    def wlay(w):
        # [1024, H] -> [128(p), 8(c), H] with row (c*128+p) at [p, c]
        return np.ascontiguousarray(
            w.reshape(8, 128, H).transpose(1, 0, 2).astype(bf))

    shared = {}
    for j in range(N):
        shared[f"wgl{j}"] = wlay(Ws[j] @ Wl1)
        shared[f"wgr{j}"] = wlay(Ws[j] @ Wr1)
    shared["wl2"] = wlay(g("Wl2"))
    shared["wr2"] = wlay(g("Wr2"))
    shared["att1"] = g("att1").reshape(1, H).astype(bf)
    shared["att2"] = g("att2").reshape(1, H).astype(bf)

    in_maps = []
    for core in range(NCORES):
        sl = slice(core * R, (core + 1) * R)
        m = dict(shared)
        NT = R // 128
        for j in range(N):
            # [R, D] -> [NT, 128(p=d%128), 8(c=d//128), 128(b)]
            a = feats[j][sl].reshape(NT, 128, 8, 128)
            m[f"xt{j}"] = np.ascontiguousarray(
                a.transpose(0, 3, 2, 1).astype(bf))
        in_maps.append(m)
    return in_maps


def kernel(**inputs):
    Btot = int(np.asarray(inputs["text_features"]).shape[0])
    assert Btot % (NCORES * 128) == 0, f"batch {Btot} not divisible by 1024"
    R = Btot // NCORES
    if R not in _CACHED:
        _CACHED[R] = _build_nc(R)
    nc = _CACHED[R]
    in_maps = _prep(inputs, R)
    res = run_bass_kernel_spmd(nc, in_maps, core_ids=list(range(NCORES)))
    _CACHED["last_result"] = res
    out = np.concatenate([r["out"] for r in res.results], axis=0)
    return np.ascontiguousarray(out.astype(np.float32))
